# revision 1
# baseline (speedup 1.0000x reference)
"""2-layer GAT on 8 Trainium2 NeuronCores (Bass/Tile).

Sharding: nodes split into 8 shards of 6272 rows (49 blocks x 128 dst nodes).
Edges routed to the core owning their dst node and sorted by dst.  Every core
redundantly computes the dense projections (x @ W, tiny) so the gather table
is core-local; message passing (gather + segment softmax + weighted scatter)
is done only for the core's own dst shard.  Layer-1 output shards are
exchanged through the host between two launches.

Per-layer device program:
  phase A: h = x @ [W | W@Asrc | W@Adst] over all nodes -> gather table rows
           [h | asrc(f32 bits)] in HBM; shard-local adst kept in SBUF.
  phase B: per 128-dst block: one dma_gather per table half (int16 idxs),
           one-hot masks built on DVE (pair-packed 2x / tensor_scalar 4x),
           adst per edge via maskT matmuls, w = exp(leaky_relu(asrc+adst)),
           aggregation + softmax denominator via PSUM-accumulated matmuls,
           normalize + bias (+ELU for layer 1).
"""

import numpy as np
import ml_dtypes

import concourse.bass as bass
import concourse.bacc as bacc
import concourse.tile as tile
from concourse import mybir
from concourse.bass_utils import run_bass_kernel_spmd

BF16 = ml_dtypes.bfloat16

N = 50000
E = 800000
IN = 128
H1 = 4
F1 = 64
NEG = 0.2
P = 128
NCORES = 8
NB = 49                 # blocks per core
SHARD = NB * P          # 6272
NPAD = 391 * P          # 50048 padded node count
NT = 391                # global node tiles
SPLIT = 196 * P         # 25088: gather-table half boundary (int16 idx limit)
GRP = 16                # phase-A load group (even: tiles processed in pairs)

_prog_cache = {}
_ABLATE = set()


# ----------------------------------------------------------------------------
# host-side edge preprocessing (shared by both layers)
# ----------------------------------------------------------------------------

def _prep_edges(edge_index):
    src = np.concatenate([edge_index[0].astype(np.int64), np.arange(N, dtype=np.int64)])
    dst = np.concatenate([edge_index[1].astype(np.int64), np.arange(N, dtype=np.int64)])
    order = np.argsort(dst, kind="stable")
    s = src[order]
    d = dst[order]

    gb = d >> 7                                   # global 128-block of dst
    NGB = NCORES * NB                             # 392
    cnt = np.bincount(gb, minlength=NGB)
    starts = np.concatenate([[0], np.cumsum(cnt)])

    isB = s >= SPLIT
    cntA = np.zeros(NGB, np.int64)
    for g in range(NGB):
        cntA[g] = np.count_nonzero(~isB[starts[g]:starts[g + 1]])
    cntB = cnt - cntA
    TA = -(-cntA // P)
    TB = -(-cntB // P)
    TAm = TA.reshape(NCORES, NB).max(0)           # cross-core max tiles, half A
    TBm = TB.reshape(NCORES, NB).max(0)
    TAm = np.maximum(TAm, 1)
    TBm = np.maximum(TBm, 1)

    Ttot = int((TAm + TBm).sum())
    Stot = Ttot * 8

    TMX = int((TAm + TBm).max())
    idx_all = np.full((NCORES, P, Stot), -1, np.int16)
    cnt_all = np.ones((NCORES, NB, 2), np.int32)
    dstl = np.full((NCORES, Ttot, P), -1.0, np.float32)   # [t, p] layout

    toffA = np.zeros(NB, np.int64)
    toffB = np.zeros(NB, np.int64)
    off = 0
    for b in range(NB):
        toffA[b] = off
        off += TAm[b]
        toffB[b] = off
        off += TBm[b]
    assert off == Ttot

    for m in range(NCORES):
        for b in range(NB):
            g = m * NB + b
            e0, e1 = starts[g], starts[g + 1]
            sb = s[e0:e1]
            db = d[e0:e1]
            mB = isB[e0:e1]
            base = m * SHARD + b * P
            for half, toff, Tmax in ((0, toffA[b], TAm[b]), (1, toffB[b], TBm[b])):
                if half == 0:
                    sh = sb[~mB]
                    dl = db[~mB] - base
                else:
                    sh = sb[mB] - SPLIT
                    dl = db[mB] - base
                npad = Tmax * P
                idxf = np.zeros(npad, np.int16)
                idxf[:len(sh)] = sh
                cnt_all[m, b, half] = max(len(sh), 1)
                dlf = np.full(npad, -1.0, np.float32)
                dlf[:len(dl)] = dl
                S = npad // 16
                w = idxf.reshape(S, 16).T
                soff = toff * 8
                idx_all[m][:, soff:soff + S] = np.tile(w, (8, 1))
                dstl[m][toff:toff + Tmax, :] = dlf.reshape(Tmax, P)

    dstl_pt = np.ascontiguousarray(dstl.transpose(0, 2, 1))        # [m, P, Ttot]
    # per-block row-major dst-locals padded to Tmax tiles: [m, NB, Tmax*P]
    dstl_row = np.full((NCORES, NB, TMX * P), -1.0, np.float32)
    for m in range(NCORES):
        for b in range(NB):
            T = TAm[b] + TBm[b]
            dstl_row[m, b, :T * P] = dstl[m, toffA[b]:toffA[b] + T].reshape(-1)
    dstl_row = dstl_row.astype(BF16)
    meta = dict(TAm=TAm.tolist(), TBm=TBm.tolist(),
                toffA=toffA.tolist(), toffB=toffB.tolist(), Ttot=Ttot, Stot=Stot,
                Tmax=TMX)
    return meta, idx_all, dstl_pt, dstl_row, cnt_all


# ----------------------------------------------------------------------------
# program builder
# ----------------------------------------------------------------------------

def _build_proj(meta):
    """Launch 0: project own shard -> [h | asrc f32-bits] table slice."""
    dt = mybir.dt
    KCH, H = 1, H1
    COUT = H1 * F1
    RCA = COUT + H                      # [h | asrc]
    ACOL = COUT + 2 * H
    nc = bacc.Bacc("TRN2", target_bir_lowering=False, debug=False,
                   num_devices=NCORES)
    xs = nc.dram_tensor("xs", [KCH, P, NB, P], dt.bfloat16,
                        kind="ExternalInput")
    wr = nc.dram_tensor("wr", [KCH, P, RCA], dt.bfloat16,
                        kind="ExternalInput")
    hts = nc.dram_tensor("hts", [SHARD, ACOL], dt.bfloat16,
                         kind="ExternalOutput")
    with tile.TileContext(nc) as tc:
        with (
            tc.tile_pool(name="const", bufs=1) as cp,
            tc.tile_pool(name="pa", bufs=3) as pa,
            tc.tile_pool(name="psA", bufs=3, space="PSUM") as psA,
        ):
            wr_sb = cp.tile([P, KCH, RCA], dt.bfloat16)
            nc.sync.dma_start(wr_sb[:], wr[:].rearrange("k p c -> p k c"))
            for g0 in range(0, NB, GRP):
                gn = min(GRP, NB - g0)
                xa = pa.tile([P, KCH, gn, P], dt.bfloat16, tag="xa")
                nc.sync.dma_start(
                    xa[:], xs[:, :, g0:g0 + gn, :].rearrange(
                        "k f t n -> f k t n"))
                hst = pa.tile([P, gn, ACOL], dt.bfloat16, tag="hst")
                for t0 in range(0, gn, 2):
                    pn = min(2, gn - t0)
                    ps = psA.tile([P, 2, 512], dt.float32, tag="psa")
                    for t2 in range(pn):
                        for k in range(KCH):
                            nc.tensor.matmul(ps[:, t2, 0:RCA],
                                             lhsT=xa[:, k, t0 + t2, :],
                                             rhs=wr_sb[:, k, 0:RCA],
                                             start=(k == 0),
                                             stop=(k == KCH - 1))
                    nc.scalar.activation(
                        hst[:, t0:t0 + pn, 0:COUT], ps[:, 0:pn, 0:COUT],
                        mybir.ActivationFunctionType.Copy)
                    nc.vector.tensor_copy(
                        hst[:, t0:t0 + pn, COUT:COUT + 2 * H].bitcast(
                            dt.float32),
                        ps[:, 0:pn, COUT:COUT + H])
                nc.sync.dma_start(
                    hts[g0 * P:(g0 + gn) * P, :].rearrange(
                        "(t n) c -> n t c", t=gn),
                    hst[:])
    nc.compile()
    return nc


def _build_layer(meta, layer, ra=1, rb=1):
    """layer 1: in 128 (1 chunk), heads 4, F 64, out bf16 [SHARD,256], elu.
    layer 2: in 256 (2 chunks), heads 1, F 64, out f32 [SHARD,64]."""
    dt = mybir.dt
    TAm, TBm = meta["TAm"], meta["TBm"]
    toffA, toffB = meta["toffA"], meta["toffB"]
    Ttot, Stot, Tmax = meta["Ttot"], meta["Stot"], meta["Tmax"]

    if layer == 1:
        KCH, H, F = 1, H1, F1
        out_dt = dt.bfloat16
    else:
        KCH, H, F = 2, 1, F1
        out_dt = dt.float32
    COUT = H * F
    RC = COUT + 2 * H                 # wr cols: [h | asrc | adst]
    RCA = COUT + H                    # phase-A cols (adst col only in A2)
    AGC = COUT + H                    # aggregation psum cols: [num | den]
    # table row layout (bf16 slots): [h (COUT) | asrc f32 bits (2H) | pad]
    ACOL = COUT + 2 * H               # used slots per row
    TABC = 384 if layer == 1 else 128  # row slots (x2 bytes, %256B)

    nc = bacc.Bacc("TRN2", target_bir_lowering=False, debug=False,
                   num_devices=NCORES)

    if layer == 1:
        w2r = nc.dram_tensor("w2r", [2, P, 65], dt.bfloat16,
                             kind="ExternalInput")
        identT = nc.dram_tensor("identT", [P, P], dt.bfloat16,
                                kind="ExternalInput")
        outT2 = nc.dram_tensor("outT2", [SHARD, 128], dt.bfloat16,
                               kind="ExternalOutput")
    xs = nc.dram_tensor("xs", [KCH, P, NB, P], dt.bfloat16, kind="ExternalInput")
    wr = nc.dram_tensor("wr", [KCH, P, RC], dt.bfloat16, kind="ExternalInput")
    idxT = nc.dram_tensor("idxT", [P, Stot], dt.int16, kind="ExternalInput")
    dstlT = nc.dram_tensor("dstlT", [P, Ttot], dt.bfloat16, kind="ExternalInput")
    dstlR = nc.dram_tensor("dstlR", [NB, Tmax * P], dt.bfloat16,
                           kind="ExternalInput")
    cntT = nc.dram_tensor("cntT", [1, NB * 2], dt.int32, kind="ExternalInput")
    brow = nc.dram_tensor("brow", [1, COUT], out_dt, kind="ExternalInput")
    iot_r = nc.dram_tensor("iot_r", [1, P], dt.bfloat16, kind="ExternalInput")
    iot_c = nc.dram_tensor("iot_c", [P, 1], dt.float32, kind="ExternalInput")

    # both layers receive the pre-assembled gather table (layer 1's from
    # the shard-projection launch, layer 2's from layer 1's fused output);
    # gathers source it directly from HBM
    htab = nc.dram_tensor("htab", [NPAD, TABC], dt.bfloat16,
                          kind="ExternalInput")
    outT = nc.dram_tensor("outT", [SHARD, COUT], out_dt, kind="ExternalOutput")
    import os as _os
    DBG = int(_os.environ.get("GAT_DBG_BLOCK", "-1"))
    if DBG >= 0:
        dbg_g = nc.dram_tensor("dbg_g", [P, Tmax * TABC], dt.bfloat16,
                               kind="ExternalOutput")
        dbg_et = nc.dram_tensor("dbg_et", [P, Tmax * H], dt.float32,
                                kind="ExternalOutput")
        dbg_hp = nc.dram_tensor("dbg_hp", [P, Tmax * AGC], dt.bfloat16,
                                kind="ExternalOutput")
        dbg_ae = nc.dram_tensor("dbg_ae", [P, Ttot * H], dt.float32,
                                kind="ExternalOutput")

    import os as _os2
    SP = bool(int(_os2.environ.get("GAT_SP", "0")))
    PBB = int(_os2.environ.get("GAT_PBB", "3"))
    GRPL = int(_os2.environ.get("GAT_GRP", "0")) or GRP
    ngrp = -(-NT // GRPL)
    # spread the NB pre-pass blocks across the phase-A groups
    grp_blocks = [[] for _ in range(ngrp)]
    for b in range(NB):
        grp_blocks[min(b * ngrp // NB, ngrp - 1)].append(b)

    with tile.TileContext(nc) as tc:
        with (
            tc.tile_pool(name="const", bufs=1) as cp,
            tc.tile_pool(name="keep", bufs=1) as kp,
            tc.tile_pool(name="pa", bufs=3) as pa,
            tc.tile_pool(name="pp", bufs=2) as pp,
            tc.tile_pool(name="pb", bufs=PBB) as pb,
            tc.tile_pool(name="sm", bufs=3) as sm,
            tc.tile_pool(name="psA", bufs=2, space="PSUM") as psA,
            tc.tile_pool(name="psB", bufs=2, space="PSUM") as psB,
        ):
            # ---- resident constants ----
            wr_sb = cp.tile([P, KCH, RC], dt.bfloat16)
            nc.sync.dma_start(wr_sb[:], wr[:].rearrange("k p c -> p k c"))
            b_sb = cp.tile([P, COUT], out_dt)
            nc.sync.dma_start(b_sb[:], brow[:].broadcast_to([P, COUT]))
            ior_sb = cp.tile([P, P], dt.bfloat16)
            nc.sync.dma_start(ior_sb[:], iot_r[:].broadcast_to([P, P]))
            ioc_sb = cp.tile([P, 1], dt.float32)
            nc.sync.dma_start(ioc_sb[:], iot_c[:])
            if layer == 1:
                w2_sb = cp.tile([P, 2, 65], dt.bfloat16)
                nc.sync.dma_start(w2_sb[:], w2r[:].rearrange("k p c -> p k c"))
                id_sb = cp.tile([P, P], dt.bfloat16)
                nc.sync.dma_start(id_sb[:], identT[:])
            idx_sb = kp.tile([P, Stot], dt.int16)
            nc.sync.dma_start(idx_sb[:], idxT[:])
            dstl_sb = kp.tile([P, Ttot], dt.bfloat16)
            nc.sync.dma_start(dstl_sb[:], dstlT[:])
            adst_sh = kp.tile([P, NB * H], dt.bfloat16)
            adst_e = kp.tile([P, Ttot * H], dt.float32)   # per-edge adst

            # ---- phase A2 first: shard-local adst ----
            for b0 in range(0, NB, 16):
                bn = min(16, NB - b0)
                xb = pa.tile([P, KCH, bn, P], dt.bfloat16, tag="xa")
                nc.sync.dma_start(
                    xb[:], xs[:, :, b0:b0 + bn, :].rearrange(
                        "k f t n -> f k t n"))
                pq = psA.tile([P, bn * H], dt.float32, tag="psq", bufs=1)
                for t in range(bn):
                    for k in range(KCH):
                        nc.tensor.matmul(pq[:, t * H:(t + 1) * H],
                                         lhsT=xb[:, k, t, :],
                                         rhs=wr_sb[:, k, COUT + H:RC],
                                         start=(k == 0), stop=(k == KCH - 1))
                nc.vector.tensor_copy(adst_sh[:, b0 * H:(b0 + bn) * H], pq[:])

            def prepass_block(b):
                T = TAm[b] + TBm[b]
                dlr = pp.tile([P, T * P], dt.bfloat16, tag="dlr")
                pbm = int(_os2.environ.get("GAT_PB_MOD", "2"))
                if pbm and b % pbm:
                    dlrow = pp.tile([1, T * P], dt.bfloat16, tag="dlrow")
                    nc.sync.dma_start(dlrow[:], dstlR[b:b + 1, 0:T * P])
                    nc.gpsimd.partition_broadcast(dlr[:], dlrow[:])
                else:
                    nc.sync.dma_start(
                        dlr[:],
                        dstlR[b:b + 1, 0:T * P].broadcast_to([P, T * P]))
                mT = pp.tile([P, T, P], dt.bfloat16, tag="mT")
                nc.vector.tensor_scalar(
                    mT[:].rearrange("p t e -> p (t e)"), dlr[:],
                    ioc_sb[:], None, mybir.AluOpType.is_equal)
                ap_ps = psB.tile([P, T * H], dt.float32, tag="adps", bufs=1)
                for t in range(T):
                    nc.tensor.matmul(ap_ps[:, t * H:(t + 1) * H],
                                     lhsT=mT[:, t, :],
                                     rhs=adst_sh[:, b * H:(b + 1) * H],
                                     start=True, stop=True)
                nc.vector.tensor_copy(
                    adst_e[:, toffA[b] * H:(toffA[b] + T) * H], ap_ps[:])

            # ---- phase A (projection sweep, layer 1 only) interleaved with
            # the pre-pass that expands adst to per-edge values ----
            for _ra in range(0):
                for gi, g0 in enumerate(range(0, NT, GRPL)):
                    gn = min(GRPL, NT - g0)
                    xa = pa.tile([P, KCH, gn, P], dt.bfloat16, tag="xa")
                    nc.sync.dma_start(
                        xa[:], xt[:, :, g0:g0 + gn, :].rearrange(
                            "k f t n -> f k t n"))
                    hst = pa.tile([P, gn, ACOL], dt.bfloat16, tag="hst")
                    for t0 in range(0, gn, 2):
                        pn = min(2, gn - t0)
                        # 512-f32 stride: each pair member in its own bank
                        ps = psA.tile([P, 2, 512], dt.float32, tag="psa")
                        for t2 in range(pn):
                            for k in range(KCH):
                                nc.tensor.matmul(ps[:, t2, 0:RCA],
                                                 lhsT=xa[:, k, t0 + t2, :],
                                                 rhs=wr_sb[:, k, 0:RCA],
                                                 start=(k == 0),
                                                 stop=(k == KCH - 1))
                        nc.scalar.activation(
                            hst[:, t0:t0 + pn, 0:COUT],
                            ps[:, 0:pn, 0:COUT],
                            mybir.ActivationFunctionType.Copy)
                        nc.vector.tensor_copy(
                            hst[:, t0:t0 + pn, COUT:COUT + 2 * H].bitcast(
                                dt.float32),
                            ps[:, 0:pn, COUT:COUT + H])
                    nc.sync.dma_start(
                        htab[g0 * P:(g0 + gn) * P, 0:ACOL].rearrange(
                            "(t n) c -> n t c", t=gn),
                        hst[:])

                    if _ra:
                        continue
                    for b in grp_blocks[gi]:
                        prepass_block(b)

            # ---- phase B: per dst-block message passing, software-pipelined
            # (gather + mask build one block ahead; adst pre-pass three) ----
            htabA = htab[0:SPLIT, :]
            htabB = htab[SPLIT:NPAD, :]

            def issue_front(b):
                TA, TB_ = TAm[b], TBm[b]
                T = TA + TB_
                g = pb.tile([P, T, TABC], dt.bfloat16, tag="gath")
                niA = TA * P
                nc.gpsimd.dma_gather(
                    g[:, 0:TA, :], htabA,
                    idx_sb[:, toffA[b] * 8:toffA[b] * 8 + TA * 8],
                    niA, niA, TABC, single_packet=SP)
                niB = TB_ * P
                nc.gpsimd.dma_gather(
                    g[:, TA:T, :], htabB,
                    idx_sb[:, toffB[b] * 8:toffB[b] * 8 + TB_ * 8],
                    niB, niB, TABC, single_packet=SP)

                # dst one-hot masks (pair-packed for DVE 2x)
                dl2 = sm.tile([P, T, 2], dt.bfloat16, tag="dl2")
                nc.vector.tensor_copy(
                    dl2[:],
                    dstl_sb[:, toffA[b]:toffA[b] + T].rearrange(
                        "p (t o) -> p t o", o=1).broadcast_to([P, T, 2]))
                mk = sm.tile([P, T, P], dt.bfloat16, tag="mk")   # [e_p,(t,d)]
                nc.vector.tensor_tensor(
                    mk[:].rearrange("p t (d2 pr) -> p t d2 pr", pr=2),
                    ior_sb[:].rearrange("p (t d2 pr) -> p t d2 pr", t=1, pr=2
                                        ).broadcast_to([P, T, P // 2, 2]),
                    dl2[:].rearrange("p t (d2 pr) -> p t d2 pr", d2=1
                                     ).broadcast_to([P, T, P // 2, 2]),
                    mybir.AluOpType.is_equal)
                return g, mk

            def epilogue(b, agg):
                # out = num/(den+eps) + b   (+ELU for layer 1)
                dn = sm.tile([P, H], dt.float32, tag="dn")
                nc.vector.tensor_scalar_add(dn[:], agg[:, COUT:AGC], 1e-16)
                rc = sm.tile([P, H], dt.float32, tag="rc")
                nc.vector.reciprocal(rc[:], dn[:])
                ob = sm.tile([P, COUT], out_dt, tag="ob")
                if layer == 1:
                    for h in range(H):
                        # ob_h = num_h * (1/den_h) + bias_h
                        nc.vector.scalar_tensor_tensor(
                            ob[:, h * F:(h + 1) * F],
                            agg[:, h * F:(h + 1) * F],
                            rc[:, h:h + 1],
                            b_sb[:, h * F:(h + 1) * F],
                            mybir.AluOpType.mult, mybir.AluOpType.add)
                else:
                    nc.vector.tensor_tensor(
                        ob[:].rearrange("p (h f) -> p h f", h=H),
                        agg[:, 0:COUT].rearrange("p (h f) -> p h f", h=H),
                        rc[:].rearrange("p (h o) -> p h o", o=1).broadcast_to(
                            [P, H, F]),
                        mybir.AluOpType.mult)
                    nc.vector.tensor_add(ob[:], ob[:], b_sb[:])
                if layer == 1:
                    # elu(y) = relu(y) + exp(min(y,0)) - 1, on ACT
                    r1 = sm.tile([P, COUT], out_dt, tag="r1")
                    nc.scalar.activation(r1[:], ob[:],
                                         mybir.ActivationFunctionType.Relu,
                                         scale=-1.0)      # relu(-y) = -min(y,0)
                    r2 = sm.tile([P, COUT], out_dt, tag="r2")
                    nc.scalar.activation(r2[:], r1[:],
                                         mybir.ActivationFunctionType.Exp,
                                         scale=-1.0)      # exp(min(y,0))
                    nc.scalar.activation(ob[:], ob[:],
                                         mybir.ActivationFunctionType.Relu)
                    # ob = (r2 - 1) + ob
                    nc.vector.scalar_tensor_tensor(
                        ob[:], r2[:], -1.0, ob[:],
                        mybir.AluOpType.add, mybir.AluOpType.add)
                nc.sync.dma_start(outT[b * P:(b + 1) * P, :], ob[:])
                if layer == 1:
                    # fused layer-2 table row production:
                    # h2 row = [elu(out1) @ W2 | asrc2 f32 bits]
                    ps_t = psA.tile([P, 2, P], dt.bfloat16, tag="psq",
                                    bufs=1)
                    for c in range(2):
                        nc.tensor.transpose(ps_t[:, c, :],
                                            ob[:, c * P:(c + 1) * P],
                                            id_sb[:])
                    x2T = sm.tile([P, 2, P], dt.bfloat16, tag="x2T")
                    nc.scalar.activation(x2T[:], ps_t[:],
                                         mybir.ActivationFunctionType.Copy)
                    ps2 = psA.tile([P, 65], dt.float32, tag="psq", bufs=1)
                    for c in range(2):
                        nc.tensor.matmul(ps2[:], lhsT=x2T[:, c, :],
                                         rhs=w2_sb[:, c, :],
                                         start=(c == 0), stop=(c == 1))
                    hst2 = sm.tile([P, 128], dt.bfloat16, tag="hst2")
                    nc.scalar.activation(hst2[:, 0:64], ps2[:, 0:64],
                                         mybir.ActivationFunctionType.Copy)
                    nc.vector.tensor_copy(
                        hst2[:, 64:66].bitcast(dt.float32), ps2[:, 64:65])
                    nc.sync.dma_start(outT2[b * P:(b + 1) * P, 0:66],
                                      hst2[:, 0:66])

            import os as _os3
            PPK = int(_os3.environ.get("GAT_PPK", "3"))
            for _rb in range(rb):
              for _pp in range(min(PPK, NB)):
                  prepass_block(_pp)
              front = issue_front(0)
              pend = None                     # (block, agg) awaiting epilogue
              for b in range(NB):
                TA, TB_ = TAm[b], TBm[b]
                T = TA + TB_
                g, mk = front
                if b + 1 < NB:
                    front = issue_front(b + 1)
                if b + PPK < NB:
                    prepass_block(b + PPK)

                # w = exp(leaky_relu(asrc + adst)); asrc from gathered rows
                et = sm.tile([P, T * H], dt.float32, tag="et")
                nc.vector.tensor_tensor(
                    et[:].rearrange("p (t h) -> p t h", h=H),
                    g[:, :, COUT:COUT + 2 * H].bitcast(dt.float32),
                    adst_e[:, toffA[b] * H:(toffA[b] + T) * H].rearrange(
                        "p (t h) -> p t h", h=H),
                    mybir.AluOpType.add)
                lr = sm.tile([P, T * H], dt.float32, tag="lr")
                nc.vector.tensor_scalar_mul(lr[:], et[:], NEG)
                nc.vector.tensor_tensor(lr[:], lr[:], et[:],
                                        mybir.AluOpType.max)
                wt = sm.tile([P, T * H], dt.bfloat16, tag="wt")
                nc.scalar.activation(wt[:], lr[:],
                                     mybir.ActivationFunctionType.Exp)
                wt2 = sm.tile([P, T, H, 2], dt.bfloat16, tag="wt2")
                nc.vector.tensor_copy(
                    wt2[:],
                    wt[:].rearrange("p (t h o) -> p t h o", h=H, o=1
                                    ).broadcast_to([P, T, H, 2]))

                # hp = [w * h | w]  (pair-packed 2x multiply)
                hp = sm.tile([P, T, AGC], dt.bfloat16, tag="hp")
                nc.vector.tensor_tensor(
                    hp[:, :, 0:COUT].rearrange("p t (h f2 pr) -> p t h f2 pr",
                                               h=H, pr=2),
                    g[:, :, 0:COUT].rearrange("p t (h f2 pr) -> p t h f2 pr",
                                              h=H, pr=2),
                    wt2[:].rearrange("p t (h1 h) pr -> p t h h1 pr", h1=1
                                     ).broadcast_to([P, T, H, F // 2, 2]),
                    mybir.AluOpType.mult)
                nc.vector.tensor_copy(
                    hp[:, :, COUT:AGC],
                    wt[:].rearrange("p (t h) -> p t h", h=H))

                if DBG == b:
                    nc.sync.dma_start(
                        dbg_g[:, 0:T * TABC],
                        g[:].rearrange("p t c -> p (t c)"))
                    nc.sync.dma_start(dbg_et[:, 0:T * H], et[:])
                    nc.sync.dma_start(
                        dbg_hp[:, 0:T * AGC],
                        hp[:].rearrange("p t c -> p (t c)"))
                    nc.sync.dma_start(dbg_ae[:], adst_e[:])

                # aggregation [num | den]
                agg = psB.tile([P, AGC], dt.float32, tag="agg")
                for t in range(T):
                    nc.tensor.matmul(agg[:], lhsT=mk[:, t, :],
                                     rhs=hp[:, t, :],
                                     start=(t == 0), stop=(t == T - 1))

                if pend is not None:
                    epilogue(*pend)
                pend = (b, agg)
              epilogue(*pend)

    nc.compile()
    return nc


# ----------------------------------------------------------------------------
# host-side weight packing
# ----------------------------------------------------------------------------

def _expand_att(att, H, F):
    out = np.zeros((H * F, H), np.float32)
    for h in range(H):
        out[h * F:(h + 1) * F, h] = att[h]
    return out


def _tiles_T(x, KCH):
    xt = x.reshape(NT, P, KCH, P).transpose(2, 3, 0, 1)   # k, f, t, n
    return np.ascontiguousarray(xt).astype(BF16)


def _inputs_layer(meta, x_full, W, att_src, att_dst, b, idx_all, dstl_pt,
                  dstl_row, cnt_all, layer):
    H = H1 if layer == 1 else 1
    F = F1
    KCH = 1 if layer == 1 else 2
    COUT = H * F
    out_np = BF16 if layer == 1 else np.float32

    xpad = np.zeros((NCORES * SHARD, KCH * P), np.float32)
    xpad[:N] = x_full

    Wf = np.asarray(W, np.float32)
    wasrc = Wf @ _expand_att(np.asarray(att_src, np.float32), H, F)
    wadst = Wf @ _expand_att(np.asarray(att_dst, np.float32), H, F)
    wr_np = np.concatenate([Wf, wasrc, wadst], axis=1)    # [KCH*128, RC]
    wr_np = np.ascontiguousarray(
        wr_np.reshape(KCH, P, COUT + 2 * H)).astype(BF16)

    b_np = np.asarray(b, np.float32).reshape(1, COUT).astype(out_np)
    ior = np.arange(P, dtype=np.float32).reshape(1, P).astype(BF16)
    ioc = np.arange(P, dtype=np.float32).reshape(P, 1)

    in_maps = []
    for m in range(NCORES):
        shard = xpad[m * SHARD:(m + 1) * SHARD]
        xs_np = np.ascontiguousarray(
            shard.reshape(NB, P, KCH, P).transpose(2, 3, 0, 1)
        ).astype(BF16)
        entry = {
            "xs": xs_np, "wr": wr_np,
            "idxT": idx_all[m],
            "dstlT": dstl_pt[m].astype(BF16),
            "dstlR": dstl_row[m],
            "cntT": cnt_all[m].reshape(1, NB * 2),
            "brow": b_np, "iot_r": ior, "iot_c": ioc,
        }
        in_maps.append(entry)
    return in_maps


def _assemble(results, cols):
    full = np.zeros((N, cols), np.float32)
    for m in range(NCORES):
        lo = m * SHARD
        hi = min(N, (m + 1) * SHARD)
        full[lo:hi] = results[m]["outT"][:hi - lo].astype(np.float32)
    return full


# ----------------------------------------------------------------------------
# entry point
# ----------------------------------------------------------------------------

def kernel(x, edge_index, W1, att_src1, att_dst1, b1, W2, att_src2, att_dst2,
           b2, _return_parts=False):
    x = np.asarray(x, np.float32)
    edge_index = np.asarray(edge_index)

    meta, idx_all, dstl_pt, dstl_row, cnt_all = _prep_edges(edge_index)

    key = (1, tuple(meta["TAm"]), tuple(meta["TBm"]))
    if key not in _prog_cache:
        _prog_cache[key] = _build_layer(meta, 1)
    ncA = _prog_cache[key]
    in_maps = _inputs_layer(meta, x, W1, att_src1, att_dst1, b1, idx_all,
                            dstl_pt, dstl_row, cnt_all, 1)
    # layer 1 also emits the layer-2 gather-table rows (fused projection)
    W2f = np.asarray(W2, np.float32)
    wasrc2 = W2f @ np.asarray(att_src2, np.float32).reshape(F1, 1)
    w2r_np = np.ascontiguousarray(
        np.concatenate([W2f, wasrc2], axis=1).reshape(2, P, 65)).astype(BF16)
    ident = np.eye(P, dtype=np.float32).astype(BF16)
    for mmap in in_maps:
        mmap["w2r"] = w2r_np
        mmap["identT"] = ident
    # launch 0: each core projects its own shard's layer-1 table slice
    key0 = (0,)
    if key0 not in _prog_cache:
        _prog_cache[key0] = _build_proj(meta)
    nc0 = _prog_cache[key0]
    RCA1 = H1 * F1 + H1
    in_maps0 = [{"xs": mmap["xs"],
                 "wr": np.ascontiguousarray(mmap["wr"][:, :, 0:RCA1])}
                for mmap in in_maps]
    res0 = run_bass_kernel_spmd(nc0, in_maps0, list(range(NCORES))).results
    htab1 = np.zeros((NPAD, 384), BF16)
    ACOL1 = H1 * F1 + 2 * H1
    for m in range(NCORES):
        lo = m * SHARD
        hi = min(NPAD, (m + 1) * SHARD)
        htab1[lo:hi, 0:ACOL1] = res0[m]["hts"][:hi - lo]
    for mmap in in_maps:
        mmap["htab"] = htab1
    resA = run_bass_kernel_spmd(ncA, in_maps, list(range(NCORES))).results
    h2in = _assemble(resA, H1 * F1)
    htab2 = np.zeros((NPAD, 128), BF16)
    for m in range(NCORES):
        lo = m * SHARD
        hi = min(NPAD, (m + 1) * SHARD)
        htab2[lo:hi] = resA[m]["outT2"][:hi - lo]

    key2 = (2, tuple(meta["TAm"]), tuple(meta["TBm"]))
    if key2 not in _prog_cache:
        _prog_cache[key2] = _build_layer(meta, 2)
    ncB = _prog_cache[key2]
    in_maps2 = _inputs_layer(meta, h2in, W2, att_src2, att_dst2, b2, idx_all,
                             dstl_pt, dstl_row, cnt_all, 2)
    for mmap in in_maps2:
        mmap["htab"] = htab2
    resB = run_bass_kernel_spmd(ncB, in_maps2, list(range(NCORES))).results
    out = _assemble(resB, F1)
    if _return_parts:
        return out, h2in
    return out



# revision 5
# speedup vs baseline: 1.0172x; 1.0172x over previous
"""2-layer GAT on 8 Trainium2 NeuronCores (Bass/Tile).

Sharding: the 391 dst 128-node blocks are sorted by half-A edge count and
dealt in groups of 8 to the cores (one block per core per iteration), so the
per-iteration cross-core tile maximum stays near the mean.  Edges are routed
to the core owning their dst block and laid out in shared tiles: half-A rows
(table rows < SPLIT, int16-indexable) first, padded to the iteration max,
then half-B rows in the same tile array (gather B runs first with leading
dummy indices, gather A then overwrites its region).

Per-layer device program (phase B only; projections are fused elsewhere):
  per 128-dst block: dma_gather B + A from the HBM row table
  [h | asrc f32-bits], one-hot dst masks on DVE (pair-packed 2x),
  per-edge adst via maskT matmuls (PSUM-resident), w = exp(prelu(asrc+adst))
  on ACT, weighted rows on DVE, aggregation + softmax denominator via
  PSUM-accumulated matmuls, epilogue scaling on ACT.

Launch 0 projects [h | asrc | adst] per node; layer 1's epilogue fuses the
layer-2 projection [h2 | asrc2 | adst2], so neither layer loads x at all.
Shards are exchanged through the host between launches.
"""

import os
import numpy as np
import ml_dtypes

import concourse.bass as bass
import concourse.bacc as bacc
import concourse.tile as tile
from concourse import mybir
from concourse.bass_utils import run_bass_kernel_spmd

BF16 = ml_dtypes.bfloat16

N = 50000
E = 800000
IN = 128
H1 = 4
F1 = 64
NEG = 0.2
P = 128
NCORES = 8
NB = 49                 # block iterations per core
SHARD = NB * P          # 6272 rows per core in the table
NPAD = 391 * P          # 50048 padded node count
NGB = NCORES * NB       # 392 block slots (391 real + 1 dummy)
SPLIT = 196 * P         # 25088: gather-table half boundary (int16 idx limit)
GRP = 16                # proj-launch load group

_prog_cache = {}


# ----------------------------------------------------------------------------
# host-side edge preprocessing (shared by both layers)
# ----------------------------------------------------------------------------

def _prep_edges(edge_index):
    src = np.concatenate([edge_index[0].astype(np.int64), np.arange(N, dtype=np.int64)])
    dst = np.concatenate([edge_index[1].astype(np.int64), np.arange(N, dtype=np.int64)])
    order = np.argsort(dst, kind="stable")
    s = src[order]
    d = dst[order]

    gb = d >> 7                                   # global 128-block of dst
    cnt = np.bincount(gb, minlength=NGB)
    starts = np.concatenate([[0], np.cumsum(cnt)])
    isB = s >= SPLIT
    cntA = np.zeros(NGB, np.int64)
    for g in range(NGB):
        cntA[g] = np.count_nonzero(~isB[starts[g]:starts[g + 1]])

    # deal blocks sorted by half-A count: iteration i gets ranks [8i, 8i+8)
    blk_order = np.argsort(-cntA, kind="stable")
    asg = blk_order.reshape(NB, NCORES)           # [iter, core] -> global block
    nA = cntA[asg]                                # [NB, NCORES]
    nBc = (cnt - cntA)[asg]
    nAmax = nA.max(1)                             # [NB]
    nBmax = nBc.max(1)
    Tm = np.maximum(-(-(nAmax + nBmax) // P), 1).astype(np.int64)
    niA16 = (-(-nAmax // 16) * 16).astype(np.int64)   # static gather-A num_idxs
    fA = nAmax // P                               # full A tiles
    rA = nAmax % P                                # B's leading dummy count
    niB = (Tm - fA) * P                           # static gather-B num_idxs

    toff = np.zeros(NB + 1, np.int64)
    np.cumsum(Tm, out=toff[1:])
    Ttot = int(toff[NB])
    TMX = int(Tm.max())

    scol = np.zeros(NB + 1, np.int64)             # idx column offsets (per 16)
    np.cumsum(niA16 // 16 + niB // 16, out=scol[1:])
    Stot = int(scol[NB])

    idx_all = np.zeros((NCORES, P, Stot), np.int16)
    dstl = np.full((NCORES, Ttot, P), -1.0, np.float32)   # [t, p] layout

    for i in range(NB):
        sA = int(scol[i])
        sB = sA + int(niA16[i] // 16)
        for m in range(NCORES):
            g = asg[i, m]
            e0, e1 = starts[g], starts[g + 1]
            sb = s[e0:e1]
            mB = isB[e0:e1]
            shA = sb[~mB]
            shB = sb[mB] - SPLIT
            dlA = (d[e0:e1][~mB] - (g << 7)).astype(np.float32)
            dlB = (d[e0:e1][mB] - (g << 7)).astype(np.float32)
            na, nb_ = len(shA), len(shB)
            # gather-A idx: real | dummy-0 to nAmax | -1 tail to niA16
            ia = np.zeros(int(niA16[i]), np.int16)
            ia[:na] = shA
            ia[int(nAmax[i]):] = -1
            # gather-B idx: rA dummy-0 | real | dummy-0 tail
            ib = np.zeros(int(niB[i]), np.int16)
            ib[int(rA[i]):int(rA[i]) + nb_] = shB
            for seg, off in ((ia, sA), (ib, sB)):
                w = seg.reshape(-1, 16).T          # [16, S]
                idx_all[m][:, off:off + w.shape[1]] = np.tile(w, (8, 1))
            dl = np.full(int(Tm[i]) * P, -1.0, np.float32)
            dl[:na] = dlA
            dl[int(nAmax[i]):int(nAmax[i]) + nb_] = dlB
            dstl[m][toff[i]:toff[i] + Tm[i], :] = dl.reshape(int(Tm[i]), P)

    dstl_pt = np.ascontiguousarray(dstl.transpose(0, 2, 1))   # [m, P, Ttot]
    dstl_row = np.full((NCORES, NB, TMX * P), -1.0, np.float32)
    for m in range(NCORES):
        for i in range(NB):
            T = int(Tm[i])
            dstl_row[m, i, :T * P] = dstl[m, toff[i]:toff[i] + T].reshape(-1)
    dstl_row = dstl_row.astype(BF16)

    meta = dict(Tm=Tm.tolist(), toff=toff.tolist(), fA=fA.tolist(),
                rA=rA.tolist(), niA16=niA16.tolist(), niB=niB.tolist(),
                scol=scol.tolist(), Ttot=Ttot, Stot=Stot, Tmax=TMX,
                asg=asg.tolist())
    return meta, idx_all, dstl_pt, dstl_row


# ----------------------------------------------------------------------------
# launch 0: project own shard -> [h | asrc | adst] table slice
# ----------------------------------------------------------------------------

def _build_proj():
    dt = mybir.dt
    KCH, H = 1, H1
    COUT = H1 * F1
    RC = COUT + 2 * H                   # [h | asrc | adst]
    OCOL = COUT + 4 * H                 # bf16 slots: h | asrc bits | adst bits
    nc = bacc.Bacc("TRN2", target_bir_lowering=False, debug=False,
                   num_devices=NCORES)
    xs = nc.dram_tensor("xs", [KCH, P, NB, P], dt.bfloat16,
                        kind="ExternalInput")
    wr = nc.dram_tensor("wr", [KCH, P, RC], dt.bfloat16,
                        kind="ExternalInput")
    hts = nc.dram_tensor("hts", [SHARD, OCOL], dt.bfloat16,
                         kind="ExternalOutput")
    with tile.TileContext(nc) as tc:
        with (
            tc.tile_pool(name="const", bufs=1) as cp,
            tc.tile_pool(name="pa", bufs=3) as pa,
            tc.tile_pool(name="psA", bufs=3, space="PSUM") as psA,
        ):
            wr_sb = cp.tile([P, KCH, RC], dt.bfloat16)
            nc.sync.dma_start(wr_sb[:], wr[:].rearrange("k p c -> p k c"))
            for g0 in range(0, NB, GRP):
                gn = min(GRP, NB - g0)
                xa = pa.tile([P, KCH, gn, P], dt.bfloat16, tag="xa")
                nc.sync.dma_start(
                    xa[:], xs[:, :, g0:g0 + gn, :].rearrange(
                        "k f t n -> f k t n"))
                hst = pa.tile([P, gn, OCOL], dt.bfloat16, tag="hst")
                for t0 in range(0, gn, 2):
                    pn = min(2, gn - t0)
                    ps = psA.tile([P, 2, 512], dt.float32, tag="psa")
                    for t2 in range(pn):
                        for k in range(KCH):
                            nc.tensor.matmul(ps[:, t2, 0:RC],
                                             lhsT=xa[:, k, t0 + t2, :],
                                             rhs=wr_sb[:, k, 0:RC],
                                             start=(k == 0),
                                             stop=(k == KCH - 1))
                    nc.scalar.activation(
                        hst[:, t0:t0 + pn, 0:COUT], ps[:, 0:pn, 0:COUT],
                        mybir.ActivationFunctionType.Copy)
                    nc.vector.tensor_copy(
                        hst[:, t0:t0 + pn, COUT:OCOL].bitcast(dt.float32),
                        ps[:, 0:pn, COUT:COUT + 2 * H])
                nc.sync.dma_start(
                    hts[g0 * P:(g0 + gn) * P, :].rearrange(
                        "(t n) c -> n t c", t=gn),
                    hst[:])
    nc.compile()
    return nc


# ----------------------------------------------------------------------------
# per-layer message-passing program (phase B)
# ----------------------------------------------------------------------------

def _build_layer(meta, layer, zero_bias):
    """layer 1: heads 4, F 64, fused layer-2 row production, no dense out.
    layer 2: heads 1, F 64, out f32 [SHARD, 64]."""
    dt = mybir.dt
    Tm, toff, fAm = meta["Tm"], meta["toff"], meta["fA"]
    niA16, niB, scol = meta["niA16"], meta["niB"], meta["scol"]
    Ttot, Stot, Tmax = meta["Ttot"], meta["Stot"], meta["Tmax"]

    if layer == 1:
        H, F = H1, F1
    else:
        H, F = 1, F1
    COUT = H * F
    AGC = COUT + H                    # aggregation psum cols: [num | den]
    TABC = 384 if layer == 1 else 128  # table row slots (256B granules)

    nc = bacc.Bacc("TRN2", target_bir_lowering=False, debug=False,
                   num_devices=NCORES)

    if layer == 1:
        w2r = nc.dram_tensor("w2r", [2, P, 66], dt.bfloat16,
                             kind="ExternalInput")
        identT = nc.dram_tensor("identT", [P, P], dt.bfloat16,
                                kind="ExternalInput")
        outT2 = nc.dram_tensor("outT2", [SHARD, 68], dt.bfloat16,
                               kind="ExternalOutput")
    else:
        outT = nc.dram_tensor("outT", [SHARD, COUT], dt.float32,
                              kind="ExternalOutput")
    idxT = nc.dram_tensor("idxT", [P, Stot], dt.int16, kind="ExternalInput")
    dstlT = nc.dram_tensor("dstlT", [P, Ttot], dt.bfloat16, kind="ExternalInput")
    dstlR = nc.dram_tensor("dstlR", [NB, Tmax * P], dt.bfloat16,
                           kind="ExternalInput")
    adstT = nc.dram_tensor("adstT", [P, NB * H], dt.bfloat16,
                           kind="ExternalInput")
    brow = nc.dram_tensor("brow", [1, COUT], dt.float32, kind="ExternalInput")
    iot_r = nc.dram_tensor("iot_r", [1, P], dt.bfloat16, kind="ExternalInput")
    iot_c = nc.dram_tensor("iot_c", [P, 1], dt.float32, kind="ExternalInput")
    htab = nc.dram_tensor("htab", [NPAD, TABC], dt.bfloat16,
                          kind="ExternalInput")

    SP = bool(int(os.environ.get("GAT_SP", "0")))
    PBB = int(os.environ.get("GAT_PBB", "3"))
    PPK = int(os.environ.get("GAT_PPK", "3"))
    PBM = int(os.environ.get("GAT_PB_MOD", "2"))

    with tile.TileContext(nc) as tc:
        with (
            tc.tile_pool(name="const", bufs=1) as cp,
            tc.tile_pool(name="keep", bufs=1) as kp,
            tc.tile_pool(name="pp", bufs=2) as ppool,
            tc.tile_pool(name="pb", bufs=PBB) as pb,
            tc.tile_pool(name="sm", bufs=3) as sm,
            tc.tile_pool(name="psA", bufs=1, space="PSUM") as psA,
            tc.tile_pool(name="psB", bufs=2, space="PSUM") as psB,
            tc.tile_pool(name="psD", bufs=PPK + 1, space="PSUM") as psD,
        ):
            # ---- resident constants ----
            b_sb = cp.tile([P, COUT], dt.float32)
            nc.sync.dma_start(b_sb[:], brow[:].broadcast_to([P, COUT]))
            ior_sb = cp.tile([P, P], dt.bfloat16)
            nc.sync.dma_start(ior_sb[:], iot_r[:].broadcast_to([P, P]))
            ioc_sb = cp.tile([P, 1], dt.float32)
            nc.sync.dma_start(ioc_sb[:], iot_c[:])
            if layer == 1:
                w2_sb = cp.tile([P, 2, 66], dt.bfloat16)
                nc.sync.dma_start(w2_sb[:], w2r[:].rearrange("k p c -> p k c"))
                id_sb = cp.tile([P, P], dt.bfloat16)
                nc.sync.dma_start(id_sb[:], identT[:])
            idx_sb = kp.tile([P, Stot], dt.int16)
            nc.sync.dma_start(idx_sb[:], idxT[:])
            dstl_sb = kp.tile([P, Ttot], dt.bfloat16)
            nc.sync.dma_start(dstl_sb[:], dstlT[:])
            adst_sh = kp.tile([P, NB * H], dt.bfloat16)
            nc.sync.dma_start(adst_sh[:], adstT[:])

            # ---- pre-pass: expand adst to per-edge values (PSUM-resident) ---
            adst_ps = [None] * NB

            def prepass_block(b):
                T = Tm[b]
                dlr = ppool.tile([P, T * P], dt.bfloat16, tag="dlr")
                if PBM and b % PBM:
                    dlrow = ppool.tile([1, T * P], dt.bfloat16, tag="dlrow")
                    nc.sync.dma_start(dlrow[:], dstlR[b:b + 1, 0:T * P])
                    nc.gpsimd.partition_broadcast(dlr[:], dlrow[:])
                else:
                    nc.sync.dma_start(
                        dlr[:],
                        dstlR[b:b + 1, 0:T * P].broadcast_to([P, T * P]))
                mT = ppool.tile([P, T, P], dt.bfloat16, tag="mT")
                nc.vector.tensor_scalar(
                    mT[:].rearrange("p t e -> p (t e)"), dlr[:],
                    ioc_sb[:], None, mybir.AluOpType.is_equal)
                ap_ps = psD.tile([P, T * H], dt.float32, tag="adps")
                for t in range(T):
                    nc.tensor.matmul(ap_ps[:, t * H:(t + 1) * H],
                                     lhsT=mT[:, t, :],
                                     rhs=adst_sh[:, b * H:(b + 1) * H],
                                     start=True, stop=True)
                adst_ps[b] = ap_ps

            # ---- gathers: B first (covers tail incl. boundary dummies),
            # then A overwrites its region ----
            htabA = htab[0:SPLIT, :]
            htabB = htab[SPLIT:NPAD, :]

            def issue_front(b):
                T = Tm[b]
                g = pb.tile([P, T, TABC], dt.bfloat16, tag="gath")
                sA = scol[b]
                sB = sA + niA16[b] // 16
                if niB[b] > 0:
                    nc.gpsimd.dma_gather(
                        g[:, fAm[b]:T, :], htabB,
                        idx_sb[:, sB:sB + niB[b] // 16],
                        niB[b], niB[b], TABC, single_packet=SP)
                if niA16[b] > 0:
                    a_tiles = -(-niA16[b] // P)
                    nc.gpsimd.dma_gather(
                        g[:, 0:a_tiles, :], htabA,
                        idx_sb[:, sA:sA + niA16[b] // 16],
                        niA16[b], niA16[b], TABC, single_packet=SP)

                # dst one-hot masks (pair-packed for DVE 2x)
                dl2 = sm.tile([P, T, 2], dt.bfloat16, tag="dl2")
                nc.vector.tensor_copy(
                    dl2[:],
                    dstl_sb[:, toff[b]:toff[b] + T].rearrange(
                        "p (t o) -> p t o", o=1).broadcast_to([P, T, 2]))
                mk = sm.tile([P, T, P], dt.bfloat16, tag="mk")   # [e_p,(t,d)]
                nc.vector.tensor_tensor(
                    mk[:].rearrange("p t (d2 pr) -> p t d2 pr", pr=2),
                    ior_sb[:].rearrange("p (t d2 pr) -> p t d2 pr", t=1, pr=2
                                        ).broadcast_to([P, T, P // 2, 2]),
                    dl2[:].rearrange("p t (d2 pr) -> p t d2 pr", d2=1
                                     ).broadcast_to([P, T, P // 2, 2]),
                    mybir.AluOpType.is_equal)
                return g, mk

            def epilogue(b, agg):
                # out = num/(den+eps) (+bias) (+ELU and fused proj, layer 1)
                dn = sm.tile([P, H], dt.float32, tag="dn")
                nc.vector.tensor_scalar_add(dn[:], agg[:, COUT:AGC], 1e-16)
                rc = sm.tile([P, H], dt.float32, tag="rc")
                nc.vector.reciprocal(rc[:], dn[:])
                if layer == 1:
                    ob = sm.tile([P, COUT], dt.bfloat16, tag="ob")
                    for h in range(H):
                        nc.scalar.activation(ob[:, h * F:(h + 1) * F],
                                             agg[:, h * F:(h + 1) * F],
                                             mybir.ActivationFunctionType.Copy,
                                             scale=rc[:, h:h + 1])
                    if not zero_bias:
                        nc.vector.tensor_add(
                            ob[:], ob[:],
                            b_sb[:].bitcast(dt.bfloat16)[:, 1::2])
                    # elu(y) = relu(y) + exp(min(y,0)) - 1
                    r1 = sm.tile([P, COUT], dt.bfloat16, tag="r1")
                    nc.scalar.activation(r1[:], ob[:],
                                         mybir.ActivationFunctionType.Relu,
                                         scale=-1.0)
                    r2 = sm.tile([P, COUT], dt.bfloat16, tag="r2")
                    nc.scalar.activation(r2[:], r1[:],
                                         mybir.ActivationFunctionType.Exp,
                                         scale=-1.0)
                    nc.scalar.activation(ob[:], ob[:],
                                         mybir.ActivationFunctionType.Relu)
                    nc.vector.scalar_tensor_tensor(
                        ob[:], r2[:], -1.0, ob[:],
                        mybir.AluOpType.add, mybir.AluOpType.add)
                    # fused layer-2 row production:
                    # [elu(out1) @ [W2|wasrc2|wadst2]] -> [h2|asrc2|adst2]
                    ps_t = psA.tile([P, 2, P], dt.bfloat16, tag="pst")
                    for c in range(2):
                        nc.tensor.transpose(ps_t[:, c, :],
                                            ob[:, c * P:(c + 1) * P],
                                            id_sb[:])
                    x2T = sm.tile([P, 2, P], dt.bfloat16, tag="x2T")
                    nc.scalar.activation(x2T[:], ps_t[:],
                                         mybir.ActivationFunctionType.Copy)
                    ps2 = psA.tile([P, 66], dt.float32, tag="ps2")
                    for c in range(2):
                        nc.tensor.matmul(ps2[:], lhsT=x2T[:, c, :],
                                         rhs=w2_sb[:, c, :],
                                         start=(c == 0), stop=(c == 1))
                    hst2 = sm.tile([P, 68], dt.bfloat16, tag="hst2")
                    nc.scalar.activation(hst2[:, 0:64], ps2[:, 0:64],
                                         mybir.ActivationFunctionType.Copy)
                    nc.vector.tensor_copy(
                        hst2[:, 64:68].bitcast(dt.float32), ps2[:, 64:66])
                    nc.sync.dma_start(outT2[b * P:(b + 1) * P, :], hst2[:])
                else:
                    ob = sm.tile([P, COUT], dt.float32, tag="ob")
                    nc.scalar.activation(ob[:], agg[:, 0:COUT],
                                         mybir.ActivationFunctionType.Copy,
                                         scale=rc[:, 0:1])
                    if not zero_bias:
                        nc.vector.tensor_add(ob[:], ob[:], b_sb[:])
                    nc.sync.dma_start(outT[b * P:(b + 1) * P, :], ob[:])

            # ---- phase B: software-pipelined per-block message passing ----
            for _pp in range(min(PPK, NB)):
                prepass_block(_pp)
            front = issue_front(0)
            pend = None                     # (block, agg) awaiting epilogue
            for b in range(NB):
                T = Tm[b]
                g, mk = front
                if b + 1 < NB:
                    front = issue_front(b + 1)
                if b + PPK < NB:
                    prepass_block(b + PPK)

                # w2 = exp(prelu(asrc + adst)) pair-broadcast, on ACT
                et = sm.tile([P, T * H], dt.float32, tag="et")
                nc.vector.tensor_tensor(
                    et[:].rearrange("p (t h) -> p t h", h=H),
                    g[:, :, COUT:COUT + 2 * H].bitcast(dt.float32),
                    adst_ps[b][:].rearrange("p (t h) -> p t h", h=H),
                    mybir.AluOpType.add)
                adst_ps[b] = None
                lr = sm.tile([P, T * H], dt.float32, tag="lr")
                nc.scalar.activation(lr[:], et[:],
                                     mybir.ActivationFunctionType.Prelu,
                                     alpha=NEG)
                wt2 = sm.tile([P, T, H, 2], dt.bfloat16, tag="wt2")
                nc.scalar.activation(
                    wt2[:],
                    lr[:].rearrange("p (t h o) -> p t h o", h=H, o=1
                                    ).broadcast_to([P, T, H, 2]),
                    mybir.ActivationFunctionType.Exp)

                # hp = [w * h | w]  (pair-packed 2x multiply)
                hp = sm.tile([P, T, AGC], dt.bfloat16, tag="hp")
                nc.vector.tensor_tensor(
                    hp[:, :, 0:COUT].rearrange("p t (h f2 pr) -> p t h f2 pr",
                                               h=H, pr=2),
                    g[:, :, 0:COUT].rearrange("p t (h f2 pr) -> p t h f2 pr",
                                              h=H, pr=2),
                    wt2[:].rearrange("p t (h1 h) pr -> p t h h1 pr", h1=1
                                     ).broadcast_to([P, T, H, F // 2, 2]),
                    mybir.AluOpType.mult)
                nc.vector.tensor_copy(
                    hp[:, :, COUT:AGC],
                    wt2[:, :, :, 0])

                # aggregation [num | den]
                agg = psB.tile([P, AGC], dt.float32, tag="agg")
                for t in range(T):
                    nc.tensor.matmul(agg[:], lhsT=mk[:, t, :],
                                     rhs=hp[:, t, :],
                                     start=(t == 0), stop=(t == T - 1))

                if pend is not None:
                    epilogue(*pend)
                pend = (b, agg)
            epilogue(*pend)

    nc.compile()
    return nc


# ----------------------------------------------------------------------------
# host-side weight packing
# ----------------------------------------------------------------------------

def _expand_att(att, H, F):
    out = np.zeros((H * F, H), np.float32)
    for h in range(H):
        out[h * F:(h + 1) * F, h] = att[h]
    return out


def _inputs_layer(meta, idx_all, dstl_pt, dstl_row, b, layer):
    H = H1 if layer == 1 else 1
    COUT = H * F1
    b_np = np.asarray(b, np.float32).reshape(1, COUT)
    ior = np.arange(P, dtype=np.float32).reshape(1, P).astype(BF16)
    ioc = np.arange(P, dtype=np.float32).reshape(P, 1)
    in_maps = []
    for m in range(NCORES):
        in_maps.append({
            "idxT": idx_all[m],
            "dstlT": dstl_pt[m].astype(BF16),
            "dstlR": dstl_row[m],
            "brow": b_np, "iot_r": ior, "iot_c": ioc,
        })
    return in_maps


# ----------------------------------------------------------------------------
# entry point
# ----------------------------------------------------------------------------

def kernel(x, edge_index, W1, att_src1, att_dst1, b1, W2, att_src2, att_dst2,
           b2):
    x = np.asarray(x, np.float32)
    edge_index = np.asarray(edge_index)

    meta, idx_all, dstl_pt, dstl_row = _prep_edges(edge_index)
    asg = np.asarray(meta["asg"])                     # [NB, NCORES]

    # ---- launch 0: per-node projection [h | asrc | adst] ----
    key0 = (0,)
    if key0 not in _prog_cache:
        _prog_cache[key0] = _build_proj()
    nc0 = _prog_cache[key0]

    W1f = np.asarray(W1, np.float32)
    wasrc1 = W1f @ _expand_att(np.asarray(att_src1, np.float32), H1, F1)
    wadst1 = W1f @ _expand_att(np.asarray(att_dst1, np.float32), H1, F1)
    wr_np = np.concatenate([W1f, wasrc1, wadst1], axis=1)
    wr_np = np.ascontiguousarray(wr_np.reshape(1, P, 256 + 2 * H1)).astype(BF16)

    xpad = np.zeros((NCORES * SHARD, IN), np.float32)
    xpad[:N] = x
    in_maps0 = []
    for m in range(NCORES):
        shard = xpad[m * SHARD:(m + 1) * SHARD]
        xs_np = np.ascontiguousarray(
            shard.reshape(NB, P, 1, P).transpose(2, 3, 0, 1)).astype(BF16)
        in_maps0.append({"xs": xs_np, "wr": wr_np})
    res0 = run_bass_kernel_spmd(nc0, in_maps0, list(range(NCORES))).results

    COUT1 = H1 * F1
    htab1 = np.zeros((NPAD, 384), BF16)
    adst1 = np.zeros((N + P, H1), np.float32)         # per-node adst (layer 1)
    for m in range(NCORES):
        lo = m * SHARD
        hi = min(NPAD, (m + 1) * SHARD)
        hts = res0[m]["hts"][:hi - lo]
        htab1[lo:hi, 0:COUT1 + 2 * H1] = hts[:, 0:COUT1 + 2 * H1]
        adst1[lo:hi] = hts[:, COUT1 + 2 * H1:].copy().view(np.float32)

    # per-core adst in assigned-block order: [P, NB*H] (partition = dst local)
    def adst_input(adst_n, H):
        out = []
        for m in range(NCORES):
            a = np.zeros((NB, P, H), np.float32)
            for i in range(NB):
                g = asg[i, m]
                rows = adst_n[g * P:(g + 1) * P]
                a[i, :len(rows)] = rows
            out.append(np.ascontiguousarray(
                a.transpose(1, 0, 2).reshape(P, NB * H)).astype(BF16))
        return out

    # ---- layer 1 ----
    tkey = tuple(meta["Tm"])
    zb1 = bool(np.all(np.asarray(b1) == 0))
    key1 = (1, tkey, zb1)
    if key1 not in _prog_cache:
        _prog_cache[key1] = _build_layer(meta, 1, zb1)
    ncA = _prog_cache[key1]
    in_maps = _inputs_layer(meta, idx_all, dstl_pt, dstl_row, b1, 1)
    W2f = np.asarray(W2, np.float32)
    wasrc2 = W2f @ np.asarray(att_src2, np.float32).reshape(F1, 1)
    wadst2 = W2f @ np.asarray(att_dst2, np.float32).reshape(F1, 1)
    w2r_np = np.ascontiguousarray(
        np.concatenate([W2f, wasrc2, wadst2], axis=1).reshape(2, P, 66)
    ).astype(BF16)
    ident = np.eye(P, dtype=np.float32).astype(BF16)
    a1in = adst_input(adst1, H1)
    for m, mmap in enumerate(in_maps):
        mmap["w2r"] = w2r_np
        mmap["identT"] = ident
        mmap["htab"] = htab1
        mmap["adstT"] = a1in[m]
    resA = run_bass_kernel_spmd(ncA, in_maps, list(range(NCORES))).results

    # reassemble layer-2 table + adst2 from assigned-block outputs
    htab2 = np.zeros((NPAD, 128), BF16)
    adst2 = np.zeros((N + P, 1), np.float32)
    for m in range(NCORES):
        o2 = resA[m]["outT2"]
        for i in range(NB):
            g = asg[i, m]
            if g * P >= NPAD:
                continue
            hi = min(NPAD, (g + 1) * P) - g * P
            htab2[g * P:g * P + hi, 0:66] = o2[i * P:i * P + hi, 0:66]
            adst2[g * P:g * P + hi, 0] = (
                o2[i * P:i * P + hi, 66:68].copy().view(np.float32)[:, 0])

    # ---- layer 2 ----
    zb2 = bool(np.all(np.asarray(b2) == 0))
    key2 = (2, tkey, zb2)
    if key2 not in _prog_cache:
        _prog_cache[key2] = _build_layer(meta, 2, zb2)
    ncB = _prog_cache[key2]
    in_maps2 = _inputs_layer(meta, idx_all, dstl_pt, dstl_row, b2, 2)
    a2in = adst_input(adst2, 1)
    for m, mmap in enumerate(in_maps2):
        mmap["htab"] = htab2
        mmap["adstT"] = a2in[m]
    resB = run_bass_kernel_spmd(ncB, in_maps2, list(range(NCORES))).results

    out = np.zeros((N, F1), np.float32)
    for m in range(NCORES):
        o = resB[m]["outT"]
        for i in range(NB):
            g = asg[i, m]
            lo = g * P
            if lo >= N:
                continue
            hi = min(N, lo + P)
            out[lo:hi] = o[i * P:i * P + (hi - lo)].astype(np.float32)
    return out


# revision 9
# speedup vs baseline: 1.0507x; 1.0330x over previous
"""2-layer GAT on 8 Trainium2 NeuronCores (Bass/Tile).

Sharding: the 391 dst 128-node blocks are sorted by half-A edge count and
dealt in groups of 8 to the cores (one block per core per iteration), so the
per-iteration cross-core tile maximum stays near the mean.  Edges are routed
to the core owning their dst block and laid out in shared tiles: half-A rows
(table rows < SPLIT, int16-indexable) first, padded to the iteration max,
then half-B rows in the same tile array (gather B runs first with leading
dummy indices, gather A then overwrites its region).

Per-layer device program (phase B only; projections are fused elsewhere):
  per 128-dst block: dma_gather B + A from the HBM row table
  [h | asrc f32-bits], one-hot dst masks on DVE (pair-packed 2x),
  per-edge adst via maskT matmuls (PSUM-resident), w = exp(prelu(asrc+adst))
  on ACT, weighted rows on DVE, aggregation + softmax denominator via
  PSUM-accumulated matmuls, epilogue scaling on ACT.

Launch 0 projects [h | asrc | adst] per node; layer 1's epilogue fuses the
layer-2 projection [h2 | asrc2 | adst2], so neither layer loads x at all.
Shards are exchanged through the host between launches.
"""

import os
import numpy as np
import ml_dtypes

import concourse.bass as bass
import concourse.bacc as bacc
import concourse.tile as tile
from concourse import mybir
from concourse.bass_utils import run_bass_kernel_spmd

BF16 = ml_dtypes.bfloat16

N = 50000
E = 800000
IN = 128
H1 = 4
F1 = 64
NEG = 0.2
P = 128
NCORES = 8
NB = 49                 # block iterations per core
SHARD = NB * P          # 6272 rows per core in the table
NPAD = 391 * P          # 50048 padded node count
NGB = NCORES * NB       # 392 block slots (391 real + 1 dummy)
SPLIT = 196 * P         # 25088: gather-table half boundary (int16 idx limit)
GRP = 16                # proj-launch load group

_prog_cache = {}


# ----------------------------------------------------------------------------
# host-side edge preprocessing (shared by both layers)
# ----------------------------------------------------------------------------

def _prep_edges(edge_index):
    src = np.concatenate([edge_index[0].astype(np.int64), np.arange(N, dtype=np.int64)])
    dst = np.concatenate([edge_index[1].astype(np.int64), np.arange(N, dtype=np.int64)])
    order = np.argsort(dst, kind="stable")
    s = src[order]
    d = dst[order]

    gb = d >> 7                                   # global 128-block of dst
    cnt = np.bincount(gb, minlength=NGB)
    starts = np.concatenate([[0], np.cumsum(cnt)])
    isB = s >= SPLIT
    cntA = np.zeros(NGB, np.int64)
    for g in range(NGB):
        cntA[g] = np.count_nonzero(~isB[starts[g]:starts[g + 1]])

    # deal blocks sorted by half-A count: iteration i gets ranks [8i, 8i+8)
    blk_order = np.argsort(-cntA, kind="stable")
    asg = blk_order.reshape(NB, NCORES)           # [iter, core] -> global block
    nA = cntA[asg]                                # [NB, NCORES]
    nBc = (cnt - cntA)[asg]
    nAmax = nA.max(1)                             # [NB]
    nBmax = nBc.max(1)
    Tm = np.maximum(-(-(nAmax + nBmax) // P), 1).astype(np.int64)
    niA16 = (-(-nAmax // 16) * 16).astype(np.int64)   # static gather-A num_idxs
    fA = nAmax // P                               # full A tiles
    rA = nAmax % P                                # B's leading dummy count
    niB = (Tm - fA) * P                           # static gather-B num_idxs

    toff = np.zeros(NB + 1, np.int64)
    np.cumsum(Tm, out=toff[1:])
    Ttot = int(toff[NB])
    TMX = int(Tm.max())

    scol = np.zeros(NB + 1, np.int64)             # idx column offsets (per 16)
    np.cumsum(niA16 // 16 + niB // 16, out=scol[1:])
    Stot = int(scol[NB])

    idx_all = np.zeros((NCORES, P, Stot), np.int16)
    dstl = np.full((NCORES, Ttot, P), -1.0, np.float32)   # [t, p] layout

    for i in range(NB):
        sA = int(scol[i])
        sB = sA + int(niA16[i] // 16)
        for m in range(NCORES):
            g = asg[i, m]
            e0, e1 = starts[g], starts[g + 1]
            sb = s[e0:e1]
            mB = isB[e0:e1]
            shA = sb[~mB]
            shB = sb[mB] - SPLIT
            dlA = (d[e0:e1][~mB] - (g << 7)).astype(np.float32)
            dlB = (d[e0:e1][mB] - (g << 7)).astype(np.float32)
            na, nb_ = len(shA), len(shB)
            # gather-A idx: real | dummy-0 to nAmax | -1 tail to niA16
            ia = np.zeros(int(niA16[i]), np.int16)
            ia[:na] = shA
            ia[int(nAmax[i]):] = -1
            # gather-B idx: rA dummy-0 | real | dummy-0 tail
            ib = np.zeros(int(niB[i]), np.int16)
            ib[int(rA[i]):int(rA[i]) + nb_] = shB
            for seg, off in ((ia, sA), (ib, sB)):
                w = seg.reshape(-1, 16).T          # [16, S]
                idx_all[m][:, off:off + w.shape[1]] = np.tile(w, (8, 1))
            dl = np.full(int(Tm[i]) * P, -1.0, np.float32)
            dl[:na] = dlA
            dl[int(nAmax[i]):int(nAmax[i]) + nb_] = dlB
            dstl[m][toff[i]:toff[i] + Tm[i], :] = dl.reshape(int(Tm[i]), P)

    dstl_pt = np.ascontiguousarray(dstl.transpose(0, 2, 1))   # [m, P, Ttot]
    dstl_row = np.full((NCORES, NB, TMX * P), -1.0, np.float32)
    for m in range(NCORES):
        for i in range(NB):
            T = int(Tm[i])
            dstl_row[m, i, :T * P] = dstl[m, toff[i]:toff[i] + T].reshape(-1)
    dstl_row = dstl_row.astype(BF16)

    meta = dict(Tm=Tm.tolist(), toff=toff.tolist(), fA=fA.tolist(),
                rA=rA.tolist(), niA16=niA16.tolist(), niB=niB.tolist(),
                scol=scol.tolist(), Ttot=Ttot, Stot=Stot, Tmax=TMX,
                asg=asg.tolist())
    return meta, idx_all, dstl_pt, dstl_row


# ----------------------------------------------------------------------------
# launch 0: project own shard -> [h | asrc | adst] table slice
# ----------------------------------------------------------------------------

def _build_proj():
    dt = mybir.dt
    KCH, H = 1, H1
    COUT = H1 * F1
    RC = COUT + 2 * H                   # [h | asrc | adst]
    OCOL = COUT + 4 * H                 # bf16 slots: h | asrc bits | adst bits
    nc = bacc.Bacc("TRN2", target_bir_lowering=False, debug=False,
                   num_devices=NCORES)
    xs = nc.dram_tensor("xs", [KCH, P, NB, P], dt.bfloat16,
                        kind="ExternalInput")
    wr = nc.dram_tensor("wr", [KCH, P, RC], dt.bfloat16,
                        kind="ExternalInput")
    hts = nc.dram_tensor("hts", [SHARD, OCOL], dt.bfloat16,
                         kind="ExternalOutput")
    with tile.TileContext(nc) as tc:
        with (
            tc.tile_pool(name="const", bufs=1) as cp,
            tc.tile_pool(name="pa", bufs=3) as pa,
            tc.tile_pool(name="psA", bufs=3, space="PSUM") as psA,
        ):
            wr_sb = cp.tile([P, KCH, RC], dt.bfloat16)
            nc.sync.dma_start(wr_sb[:], wr[:].rearrange("k p c -> p k c"))
            for g0 in range(0, NB, GRP):
                gn = min(GRP, NB - g0)
                xa = pa.tile([P, KCH, gn, P], dt.bfloat16, tag="xa")
                nc.sync.dma_start(
                    xa[:], xs[:, :, g0:g0 + gn, :].rearrange(
                        "k f t n -> f k t n"))
                hst = pa.tile([P, gn, OCOL], dt.bfloat16, tag="hst")
                for t0 in range(0, gn, 2):
                    pn = min(2, gn - t0)
                    ps = psA.tile([P, 2, 512], dt.float32, tag="psa")
                    for t2 in range(pn):
                        for k in range(KCH):
                            nc.tensor.matmul(ps[:, t2, 0:RC],
                                             lhsT=xa[:, k, t0 + t2, :],
                                             rhs=wr_sb[:, k, 0:RC],
                                             start=(k == 0),
                                             stop=(k == KCH - 1))
                    nc.scalar.activation(
                        hst[:, t0:t0 + pn, 0:COUT], ps[:, 0:pn, 0:COUT],
                        mybir.ActivationFunctionType.Copy)
                    nc.vector.tensor_copy(
                        hst[:, t0:t0 + pn, COUT:OCOL].bitcast(dt.float32),
                        ps[:, 0:pn, COUT:COUT + 2 * H])
                nc.sync.dma_start(
                    hts[g0 * P:(g0 + gn) * P, :].rearrange(
                        "(t n) c -> n t c", t=gn),
                    hst[:])
    nc.compile()
    return nc


# ----------------------------------------------------------------------------
# per-layer message-passing program (phase B)
# ----------------------------------------------------------------------------

def _build_layer(meta, layer, zero_bias):
    """layer 1: heads 4, F 64, fused layer-2 row production, no dense out.
    layer 2: heads 1, F 64, out f32 [SHARD, 64]."""
    dt = mybir.dt
    Tm, toff, fAm = meta["Tm"], meta["toff"], meta["fA"]
    niA16, niB, scol = meta["niA16"], meta["niB"], meta["scol"]
    Ttot, Stot, Tmax = meta["Ttot"], meta["Stot"], meta["Tmax"]

    if layer == 1:
        H, F = H1, F1
    else:
        H, F = 1, F1
    COUT = H * F
    AGC = COUT + H                    # aggregation psum cols: [num | den]
    TABC = 384 if layer == 1 else 128  # table row slots (256B granules)

    nc = bacc.Bacc("TRN2", target_bir_lowering=False, debug=False,
                   num_devices=NCORES)

    if layer == 1:
        w2r = nc.dram_tensor("w2r", [2, P, 66], dt.bfloat16,
                             kind="ExternalInput")
        identT = nc.dram_tensor("identT", [P, P], dt.bfloat16,
                                kind="ExternalInput")
        outT2 = nc.dram_tensor("outT2", [SHARD, 68], dt.bfloat16,
                               kind="ExternalOutput")
    else:
        outT = nc.dram_tensor("outT", [SHARD, COUT], dt.float32,
                              kind="ExternalOutput")
    idxT = nc.dram_tensor("idxT", [P, Stot], dt.int16, kind="ExternalInput")
    dstlT = nc.dram_tensor("dstlT", [P, Ttot], dt.bfloat16, kind="ExternalInput")
    dstlR = nc.dram_tensor("dstlR", [NB, Tmax * P], dt.bfloat16,
                           kind="ExternalInput")
    adstT = nc.dram_tensor("adstT", [P, NB * H], dt.bfloat16,
                           kind="ExternalInput")
    brow = nc.dram_tensor("brow", [1, COUT], dt.float32, kind="ExternalInput")
    iot_r = nc.dram_tensor("iot_r", [1, P], dt.bfloat16, kind="ExternalInput")
    iot_c = nc.dram_tensor("iot_c", [P, 1], dt.float32, kind="ExternalInput")
    htab = nc.dram_tensor("htab", [NPAD, TABC], dt.bfloat16,
                          kind="ExternalInput")

    SP = bool(int(os.environ.get("GAT_SP", "0")))
    PBB = int(os.environ.get("GAT_PBB", "4"))
    PPK = int(os.environ.get("GAT_PPK", "3"))
    PBM = int(os.environ.get("GAT_PB_MOD", "2"))

    with tile.TileContext(nc) as tc:
        with (
            tc.tile_pool(name="const", bufs=1) as cp,
            tc.tile_pool(name="keep", bufs=1) as kp,
            tc.tile_pool(name="pp", bufs=3) as ppool,
            tc.tile_pool(name="pb", bufs=PBB) as pb,
            tc.tile_pool(name="sm", bufs=3) as sm,
            tc.tile_pool(name="psA", bufs=1, space="PSUM") as psA,
            tc.tile_pool(name="psB", bufs=2, space="PSUM") as psB,
            tc.tile_pool(name="psD", bufs=PPK + 1, space="PSUM") as psD,
        ):
            # ---- resident constants ----
            b_sb = cp.tile([P, COUT], dt.float32)
            nc.sync.dma_start(b_sb[:], brow[:].broadcast_to([P, COUT]))
            ior_sb = cp.tile([P, P], dt.bfloat16)
            nc.sync.dma_start(ior_sb[:], iot_r[:].broadcast_to([P, P]))
            ioc_sb = cp.tile([P, 1], dt.float32)
            nc.sync.dma_start(ioc_sb[:], iot_c[:])
            if layer == 1:
                w2_sb = cp.tile([P, 2, 66], dt.bfloat16)
                nc.sync.dma_start(w2_sb[:], w2r[:].rearrange("k p c -> p k c"))
                id_sb = cp.tile([P, P], dt.bfloat16)
                nc.sync.dma_start(id_sb[:], identT[:])
            idx_sb = kp.tile([P, Stot], dt.int16)
            nc.sync.dma_start(idx_sb[:], idxT[:])
            dstl_sb = kp.tile([P, Ttot], dt.bfloat16)
            nc.sync.dma_start(dstl_sb[:], dstlT[:])
            adst_sh = kp.tile([P, NB * H], dt.bfloat16)
            nc.sync.dma_start(adst_sh[:], adstT[:])

            # ---- pre-pass: expand adst to per-edge values (PSUM-resident),
            # staged so the dlr broadcast has a full iteration to land ----
            adst_ps = [None] * NB
            dlr_sb = [None] * NB

            def dlr_issue(b):
                T = Tm[b]
                dlr = ppool.tile([P, T * P], dt.bfloat16, tag="dlr")
                if PBM and b % PBM:
                    dlrow = ppool.tile([1, T * P], dt.bfloat16, tag="dlrow")
                    nc.sync.dma_start(dlrow[:], dstlR[b:b + 1, 0:T * P])
                    nc.gpsimd.partition_broadcast(dlr[:], dlrow[:])
                else:
                    nc.sync.dma_start(
                        dlr[:],
                        dstlR[b:b + 1, 0:T * P].broadcast_to([P, T * P]))
                dlr_sb[b] = dlr

            def prepass_block(b):
                T = Tm[b]
                mT = ppool.tile([P, T, P], dt.bfloat16, tag="mT")
                nc.vector.tensor_scalar(
                    mT[:].rearrange("p t e -> p (t e)"), dlr_sb[b][:],
                    ioc_sb[:], None, mybir.AluOpType.is_equal)
                dlr_sb[b] = None
                ap_ps = psD.tile([P, T * H], dt.float32, tag="adps")
                for t in range(T):
                    nc.tensor.matmul(ap_ps[:, t * H:(t + 1) * H],
                                     lhsT=mT[:, t, :],
                                     rhs=adst_sh[:, b * H:(b + 1) * H],
                                     start=True, stop=True)
                adst_ps[b] = ap_ps

            # ---- gathers: B first (covers tail incl. boundary dummies),
            # then A overwrites its region ----
            htabA = htab[0:SPLIT, :]
            htabB = htab[SPLIT:NPAD, :]

            g_sb = [None] * NB
            mk_sb = [None] * NB

            def issue_gatherB(b):
                T = Tm[b]
                g = pb.tile([P, T, TABC], dt.bfloat16, tag="gath")
                sB = scol[b] + niA16[b] // 16
                if niB[b] > 0:
                    nc.gpsimd.dma_gather(
                        g[:, fAm[b]:T, :], htabB,
                        idx_sb[:, sB:sB + niB[b] // 16],
                        niB[b], niB[b], TABC, single_packet=SP)
                g_sb[b] = g

            def issue_gatherA(b):
                T = Tm[b]
                g = g_sb[b]
                sA = scol[b]
                if niA16[b] > 0:
                    a_tiles = -(-niA16[b] // P)
                    nc.gpsimd.dma_gather(
                        g[:, 0:a_tiles, :], htabA,
                        idx_sb[:, sA:sA + niA16[b] // 16],
                        niA16[b], niA16[b], TABC, single_packet=SP)

            def build_masks(b):
                T = Tm[b]
                # dst one-hot masks (pair-packed for DVE 2x)
                dl2 = sm.tile([P, T, 2], dt.bfloat16, tag="dl2")
                nc.vector.tensor_copy(
                    dl2[:],
                    dstl_sb[:, toff[b]:toff[b] + T].rearrange(
                        "p (t o) -> p t o", o=1).broadcast_to([P, T, 2]))
                mk = sm.tile([P, T, P], dt.bfloat16, tag="mk")   # [e_p,(t,d)]
                nc.vector.tensor_tensor(
                    mk[:].rearrange("p t (d2 pr) -> p t d2 pr", pr=2),
                    ior_sb[:].rearrange("p (t d2 pr) -> p t d2 pr", t=1, pr=2
                                        ).broadcast_to([P, T, P // 2, 2]),
                    dl2[:].rearrange("p t (d2 pr) -> p t d2 pr", d2=1
                                     ).broadcast_to([P, T, P // 2, 2]),
                    mybir.AluOpType.is_equal)
                mk_sb[b] = mk

            def epilogue(b, agg):
                # out = num/(den+eps) (+bias) (+ELU and fused proj, layer 1)
                dn = sm.tile([P, H], dt.float32, tag="dn")
                nc.vector.tensor_scalar_add(dn[:], agg[:, COUT:AGC], 1e-16)
                rc = sm.tile([P, H], dt.float32, tag="rc")
                nc.vector.reciprocal(rc[:], dn[:])
                if layer == 1:
                    ob = sm.tile([P, COUT], dt.bfloat16, tag="ob")
                    for h in range(H):
                        nc.scalar.activation(ob[:, h * F:(h + 1) * F],
                                             agg[:, h * F:(h + 1) * F],
                                             mybir.ActivationFunctionType.Copy,
                                             scale=rc[:, h:h + 1])
                    if not zero_bias:
                        nc.vector.tensor_add(
                            ob[:], ob[:],
                            b_sb[:].bitcast(dt.bfloat16)[:, 1::2])
                    # elu(y) = relu(y) + exp(min(y,0)) - 1
                    r1 = sm.tile([P, COUT], dt.bfloat16, tag="r1")
                    nc.scalar.activation(r1[:], ob[:],
                                         mybir.ActivationFunctionType.Relu,
                                         scale=-1.0)
                    r2 = sm.tile([P, COUT], dt.bfloat16, tag="r2")
                    nc.scalar.activation(r2[:], r1[:],
                                         mybir.ActivationFunctionType.Exp,
                                         scale=-1.0)
                    nc.scalar.activation(ob[:], ob[:],
                                         mybir.ActivationFunctionType.Relu)
                    nc.vector.scalar_tensor_tensor(
                        ob[:], r2[:], -1.0, ob[:],
                        mybir.AluOpType.add, mybir.AluOpType.add)
                    # fused layer-2 row production:
                    # [elu(out1) @ [W2|wasrc2|wadst2]] -> [h2|asrc2|adst2]
                    ps_t = psA.tile([P, 2, P], dt.bfloat16, tag="pst")
                    for c in range(2):
                        nc.tensor.transpose(ps_t[:, c, :],
                                            ob[:, c * P:(c + 1) * P],
                                            id_sb[:])
                    x2T = sm.tile([P, 2, P], dt.bfloat16, tag="x2T")
                    nc.scalar.activation(x2T[:], ps_t[:],
                                         mybir.ActivationFunctionType.Copy)
                    ps2 = psA.tile([P, 66], dt.float32, tag="ps2")
                    for c in range(2):
                        nc.tensor.matmul(ps2[:], lhsT=x2T[:, c, :],
                                         rhs=w2_sb[:, c, :],
                                         start=(c == 0), stop=(c == 1))
                    hst2 = sm.tile([P, 68], dt.bfloat16, tag="hst2")
                    nc.scalar.activation(hst2[:, 0:64], ps2[:, 0:64],
                                         mybir.ActivationFunctionType.Copy)
                    nc.vector.tensor_copy(
                        hst2[:, 64:68].bitcast(dt.float32), ps2[:, 64:66])
                    nc.sync.dma_start(outT2[b * P:(b + 1) * P, :], hst2[:])
                else:
                    ob = sm.tile([P, COUT], dt.float32, tag="ob")
                    nc.scalar.activation(ob[:], agg[:, 0:COUT],
                                         mybir.ActivationFunctionType.Copy,
                                         scale=rc[:, 0:1])
                    if not zero_bias:
                        nc.vector.tensor_add(ob[:], ob[:], b_sb[:])
                    nc.sync.dma_start(outT[b * P:(b + 1) * P, :], ob[:])

            # ---- phase B: software-pipelined per-block message passing.
            # Emission order is tuned for the in-order engine queues: the
            # et->prelu->exp->hp critical chain leads, lookahead issues fill
            # the ACT round-trip, the lagged epilogue never blocks it. ----
            for q in range(min(PPK, NB)):
                dlr_issue(q)
            for q in range(min(PPK - 1, NB)):
                prepass_block(q)
            for q in range(min(2, NB)):
                issue_gatherB(q)
            issue_gatherA(0)
            build_masks(0)
            pend = None                     # (block, agg) awaiting epilogue
            for b in range(NB):
                T = Tm[b]
                g = g_sb[b]
                mk = mk_sb[b]

                # w2 = exp(prelu(asrc + adst)) pair-broadcast, on ACT
                et = sm.tile([P, T * H], dt.float32, tag="et")
                nc.vector.tensor_tensor(
                    et[:].rearrange("p (t h) -> p t h", h=H),
                    g[:, :, COUT:COUT + 2 * H].bitcast(dt.float32),
                    adst_ps[b][:].rearrange("p (t h) -> p t h", h=H),
                    mybir.AluOpType.add)
                adst_ps[b] = None
                lr = sm.tile([P, T * H], dt.float32, tag="lr")
                nc.scalar.activation(lr[:], et[:],
                                     mybir.ActivationFunctionType.Prelu,
                                     alpha=NEG)
                wt2 = sm.tile([P, T, H, 2], dt.bfloat16, tag="wt2")
                nc.scalar.activation(
                    wt2[:],
                    lr[:].rearrange("p (t h o) -> p t h o", h=H, o=1
                                    ).broadcast_to([P, T, H, 2]),
                    mybir.ActivationFunctionType.Exp)

                # lookahead issues (fill the ACT round-trip gap on DVE/Pool)
                if b + PPK < NB:
                    dlr_issue(b + PPK)
                if b + PPK - 1 < NB:
                    prepass_block(b + PPK - 1)
                if b + 2 < NB:
                    issue_gatherB(b + 2)
                if pend is not None:
                    epilogue(*pend)
                    pend = None
                if b + 1 < NB:
                    issue_gatherA(b + 1)
                    build_masks(b + 1)

                # hp = [w * h | w]  (pair-packed 2x multiply)
                hp = sm.tile([P, T, AGC], dt.bfloat16, tag="hp")
                nc.vector.tensor_tensor(
                    hp[:, :, 0:COUT].rearrange("p t (h f2 pr) -> p t h f2 pr",
                                               h=H, pr=2),
                    g[:, :, 0:COUT].rearrange("p t (h f2 pr) -> p t h f2 pr",
                                              h=H, pr=2),
                    wt2[:].rearrange("p t (h1 h) pr -> p t h h1 pr", h1=1
                                     ).broadcast_to([P, T, H, F // 2, 2]),
                    mybir.AluOpType.mult)
                nc.vector.tensor_copy(
                    hp[:, :, COUT:AGC],
                    wt2[:, :, :, 0])
                g_sb[b] = None
                mk_sb[b] = None

                # aggregation [num | den]
                agg = psB.tile([P, AGC], dt.float32, tag="agg")
                for t in range(T):
                    nc.tensor.matmul(agg[:], lhsT=mk[:, t, :],
                                     rhs=hp[:, t, :],
                                     start=(t == 0), stop=(t == T - 1))
                pend = (b, agg)
            epilogue(*pend)

    nc.compile()
    return nc


# ----------------------------------------------------------------------------
# host-side weight packing
# ----------------------------------------------------------------------------

def _expand_att(att, H, F):
    out = np.zeros((H * F, H), np.float32)
    for h in range(H):
        out[h * F:(h + 1) * F, h] = att[h]
    return out


def _inputs_layer(meta, idx_all, dstl_pt, dstl_row, b, layer):
    H = H1 if layer == 1 else 1
    COUT = H * F1
    b_np = np.asarray(b, np.float32).reshape(1, COUT)
    ior = np.arange(P, dtype=np.float32).reshape(1, P).astype(BF16)
    ioc = np.arange(P, dtype=np.float32).reshape(P, 1)
    in_maps = []
    for m in range(NCORES):
        in_maps.append({
            "idxT": idx_all[m],
            "dstlT": dstl_pt[m].astype(BF16),
            "dstlR": dstl_row[m],
            "brow": b_np, "iot_r": ior, "iot_c": ioc,
        })
    return in_maps


# ----------------------------------------------------------------------------
# entry point
# ----------------------------------------------------------------------------

def kernel(x, edge_index, W1, att_src1, att_dst1, b1, W2, att_src2, att_dst2,
           b2):
    x = np.asarray(x, np.float32)
    edge_index = np.asarray(edge_index)

    meta, idx_all, dstl_pt, dstl_row = _prep_edges(edge_index)
    asg = np.asarray(meta["asg"])                     # [NB, NCORES]

    # ---- launch 0: per-node projection [h | asrc | adst] ----
    key0 = (0,)
    if key0 not in _prog_cache:
        _prog_cache[key0] = _build_proj()
    nc0 = _prog_cache[key0]

    W1f = np.asarray(W1, np.float32)
    wasrc1 = W1f @ _expand_att(np.asarray(att_src1, np.float32), H1, F1)
    wadst1 = W1f @ _expand_att(np.asarray(att_dst1, np.float32), H1, F1)
    wr_np = np.concatenate([W1f, wasrc1, wadst1], axis=1)
    wr_np = np.ascontiguousarray(wr_np.reshape(1, P, 256 + 2 * H1)).astype(BF16)

    xpad = np.zeros((NCORES * SHARD, IN), np.float32)
    xpad[:N] = x
    in_maps0 = []
    for m in range(NCORES):
        shard = xpad[m * SHARD:(m + 1) * SHARD]
        xs_np = np.ascontiguousarray(
            shard.reshape(NB, P, 1, P).transpose(2, 3, 0, 1)).astype(BF16)
        in_maps0.append({"xs": xs_np, "wr": wr_np})
    res0 = run_bass_kernel_spmd(nc0, in_maps0, list(range(NCORES))).results

    COUT1 = H1 * F1
    htab1 = np.zeros((NPAD, 384), BF16)
    adst1 = np.zeros((N + P, H1), np.float32)         # per-node adst (layer 1)
    for m in range(NCORES):
        lo = m * SHARD
        hi = min(NPAD, (m + 1) * SHARD)
        hts = res0[m]["hts"][:hi - lo]
        htab1[lo:hi, 0:COUT1 + 2 * H1] = hts[:, 0:COUT1 + 2 * H1]
        adst1[lo:hi] = hts[:, COUT1 + 2 * H1:].copy().view(np.float32)

    # per-core adst in assigned-block order: [P, NB*H] (partition = dst local)
    def adst_input(adst_n, H):
        out = []
        for m in range(NCORES):
            a = np.zeros((NB, P, H), np.float32)
            for i in range(NB):
                g = asg[i, m]
                rows = adst_n[g * P:(g + 1) * P]
                a[i, :len(rows)] = rows
            out.append(np.ascontiguousarray(
                a.transpose(1, 0, 2).reshape(P, NB * H)).astype(BF16))
        return out

    # ---- layer 1 ----
    tkey = tuple(meta["Tm"])
    zb1 = bool(np.all(np.asarray(b1) == 0))
    key1 = (1, tkey, zb1)
    if key1 not in _prog_cache:
        _prog_cache[key1] = _build_layer(meta, 1, zb1)
    ncA = _prog_cache[key1]
    in_maps = _inputs_layer(meta, idx_all, dstl_pt, dstl_row, b1, 1)
    W2f = np.asarray(W2, np.float32)
    wasrc2 = W2f @ np.asarray(att_src2, np.float32).reshape(F1, 1)
    wadst2 = W2f @ np.asarray(att_dst2, np.float32).reshape(F1, 1)
    w2r_np = np.ascontiguousarray(
        np.concatenate([W2f, wasrc2, wadst2], axis=1).reshape(2, P, 66)
    ).astype(BF16)
    ident = np.eye(P, dtype=np.float32).astype(BF16)
    a1in = adst_input(adst1, H1)
    for m, mmap in enumerate(in_maps):
        mmap["w2r"] = w2r_np
        mmap["identT"] = ident
        mmap["htab"] = htab1
        mmap["adstT"] = a1in[m]
    resA = run_bass_kernel_spmd(ncA, in_maps, list(range(NCORES))).results

    # reassemble layer-2 table + adst2 from assigned-block outputs
    htab2 = np.zeros((NPAD, 128), BF16)
    adst2 = np.zeros((N + P, 1), np.float32)
    for m in range(NCORES):
        o2 = resA[m]["outT2"]
        for i in range(NB):
            g = asg[i, m]
            if g * P >= NPAD:
                continue
            hi = min(NPAD, (g + 1) * P) - g * P
            htab2[g * P:g * P + hi, 0:66] = o2[i * P:i * P + hi, 0:66]
            adst2[g * P:g * P + hi, 0] = (
                o2[i * P:i * P + hi, 66:68].copy().view(np.float32)[:, 0])

    # ---- layer 2 ----
    zb2 = bool(np.all(np.asarray(b2) == 0))
    key2 = (2, tkey, zb2)
    if key2 not in _prog_cache:
        _prog_cache[key2] = _build_layer(meta, 2, zb2)
    ncB = _prog_cache[key2]
    in_maps2 = _inputs_layer(meta, idx_all, dstl_pt, dstl_row, b2, 2)
    a2in = adst_input(adst2, 1)
    for m, mmap in enumerate(in_maps2):
        mmap["htab"] = htab2
        mmap["adstT"] = a2in[m]
    resB = run_bass_kernel_spmd(ncB, in_maps2, list(range(NCORES))).results

    out = np.zeros((N, F1), np.float32)
    for m in range(NCORES):
        o = resB[m]["outT"]
        for i in range(NB):
            g = asg[i, m]
            lo = g * P
            if lo >= N:
                continue
            hi = min(N, lo + P)
            out[lo:hi] = o[i * P:i * P + (hi - lo)].astype(np.float32)
    return out
